# revision 10
# baseline (speedup 1.0000x reference)
"""CRF output layer (projection + Viterbi decode) on 8 Trainium2 cores.

Self-contained: hardcodes B=128, T=1024, H=512, K=50. Shards batch over the
8 cores (16 rows each). Per core:
  - projection: PE matmuls (hidden pre-transposed host-side); feats land in
    SBUF chunk layout feats[(b,c1), r, j], r <-> t = c1*128-16+r (16 warmup
    rows + 128 body + 16 extension rows duplicated across chunks).
  - forward Viterbi values: 8 time-chunks x 16 batch rows = 128 lanes, 159
    steps of (broadcast add, segmented max over prev tag, +feat) on DVE.
    Warmup chunks coalesce to the true state (max-plus contraction).
  - backtrace: 64 windows of 16 tags (8 slots x 8 chunk groups), 32 steps;
    one-hot states drive PE gather matmuls; first-index argmax on DVE.
  - best score: re-score the decoded path, reduce, group-sum on PE.
Returns (best_score [B] f32, tags [B,T] i32) matching the reference tuple.
"""
import numpy as np

B, T, H, K = 128, 1024, 512, 50
NCORES = 8
BC = B // NCORES
C1 = 8
LF = 16
ROWS = 160
NE = 8
BIG = 1.0e6

_cache = {}


def _build():
    import concourse.bass as bass  # noqa: F401
    from concourse import bacc, mybir
    from concourse.tile import TileContext

    f32 = mybir.dt.float32
    i32 = mybir.dt.int32
    X = mybir.AxisListType.X
    XY = mybir.AxisListType.XY
    op = mybir.AluOpType

    nc = bacc.Bacc("TRN2", target_bir_lowering=False, debug=False,
                   num_devices=NCORES)

    hT = nc.dram_tensor("hT", [H, BC * T], f32, kind="ExternalInput")
    wt = nc.dram_tensor("wt", [H, K], f32, kind="ExternalInput")
    b_rep = nc.dram_tensor("b_rep", [128, K], f32, kind="ExternalInput")
    st_rep = nc.dram_tensor("st_rep", [128, K], f32, kind="ExternalInput")
    sp_rep = nc.dram_tensor("sp_rep", [128, K], f32, kind="ExternalInput")
    at_rep = nc.dram_tensor("at_rep", [128, K * K], f32, kind="ExternalInput")
    i128 = nc.dram_tensor("i128", [128, 128], f32, kind="ExternalInput")
    selr = nc.dram_tensor("selr", [128, 2 * K], f32, kind="ExternalInput")
    c1c = nc.dram_tensor("c1c", [128, NE * K], f32, kind="ExternalInput")
    iot = nc.dram_tensor("iot", [128, NE * K], f32, kind="ExternalInput")
    oh0 = nc.dram_tensor("oh0", [128, 128], f32, kind="ExternalInput")
    grp = nc.dram_tensor("grp", [128, BC], f32, kind="ExternalInput")
    m0k = nc.dram_tensor("m0k", [128, K], f32, kind="ExternalInput")
    stm = nc.dram_tensor("stm", [128, K], f32, kind="ExternalInput")
    m7o = nc.dram_tensor("m7o", [128, 1], f32, kind="ExternalInput")
    km7 = nc.dram_tensor("km7", [128, K], f32, kind="ExternalInput")

    tags_out = nc.dram_tensor("tags", [BC, T], i32, kind="ExternalOutput")
    best_out = nc.dram_tensor("best", [BC, 1], f32, kind="ExternalOutput")

    sb0, sb7 = slice(0, 121, 8), slice(7, 128, 8)   # (b,0) / (b,7) lanes

    with TileContext(nc) as tc:
        with tc.tile_pool(name="consts", bufs=1) as consts:
            c_b = consts.tile([128, K], f32)
            nc.sync.dma_start(c_b[:], b_rep.ap())
            c_st = consts.tile([128, K], f32)
            nc.sync.dma_start(c_st[:], st_rep.ap())
            c_sp = consts.tile([128, K], f32)
            nc.sync.dma_start(c_sp[:], sp_rep.ap())
            c_at = consts.tile([128, K, K], f32)
            nc.sync.dma_start(c_at[:], at_rep.ap().rearrange(
                "p (j i) -> p j i", j=K))
            c_i128 = consts.tile([128, 128], f32)
            nc.sync.dma_start(c_i128[:], i128.ap())
            c_selr = consts.tile([128, 2 * K], f32)
            nc.sync.dma_start(c_selr[:], selr.ap())
            c_c1 = consts.tile([128, NE * K], f32)
            nc.sync.dma_start(c_c1[:], c1c.ap())
            c_iot = consts.tile([128, NE * K], f32)
            nc.sync.dma_start(c_iot[:], iot.ap())
            c_wt = consts.tile([128, 4, K], f32)
            nc.sync.dma_start(c_wt[:], wt.ap().rearrange(
                "(c p) k -> p c k", p=128))
            c_grp = consts.tile([128, BC], f32)
            nc.sync.dma_start(c_grp[:], grp.ap())
            c_m0k = consts.tile([128, K], f32)
            nc.sync.dma_start(c_m0k[:], m0k.ap())
            c_stm = consts.tile([128, K], f32)
            nc.sync.dma_start(c_stm[:], stm.ap())
            c_m7o = consts.tile([128, 1], f32)
            nc.sync.dma_start(c_m7o[:], m7o.ap())
            c_km7 = consts.tile([128, K], f32)
            nc.sync.dma_start(c_km7[:], km7.ap())

            big = tc.tile_pool(name="bigp", bufs=1)
            with big as bigp:
                feats = bigp.tile([128, ROWS, K], f32)
                v_all = bigp.tile([128, ROWS, K], f32)

                nc.vector.memset(feats[:], 0.0)

                # ---------- projection ----------
                with tc.tile_pool(name="hbuf", bufs=2) as hpool, \
                     tc.tile_pool(name="fstp", bufs=4) as fpool, \
                     tc.tile_pool(name="ppsm", bufs=4, space="PSUM") as ppool:
                    for b in range(BC):
                        hb = hpool.tile([128, 4, T], f32, tag="hb")
                        nc.sync.dma_start(hb[:], hT.ap()[:, b * T:(b + 1) * T]
                                          .rearrange("(c p) t -> p c t", p=128))
                        for c in range(C1):
                            ps = ppool.tile([128, K], f32, tag="ps")
                            for hc in range(4):
                                nc.tensor.matmul(
                                    ps[:], hb[:, hc, c * 128:(c + 1) * 128],
                                    c_wt[:, hc, :],
                                    start=(hc == 0), stop=(hc == 3))
                            fs = fpool.tile([128, K], f32, tag="fs")
                            nc.vector.tensor_add(fs[:], ps[:], c_b[:])
                            p0 = b * C1 + c
                            nc.sync.dma_start(feats[p0:p0 + 1, LF:LF + 128, :],
                                              fs[:])
                            if c + 1 < C1:
                                nc.sync.dma_start(feats[p0 + 1:p0 + 2, 0:LF, :],
                                                  fs[112:128, :])
                            if c >= 1:
                                nc.sync.dma_start(
                                    feats[p0 - 1:p0, ROWS - 16:ROWS, :],
                                    fs[0:16, :])

                # ---------- forward ----------
                with tc.tile_pool(name="fwdp", bufs=2) as fwd:
                    nc.vector.tensor_copy(v_all[:, 0, :], feats[:, 0, :])
                    for r in range(1, ROWS):
                        sc = fwd.tile([128, K, K], f32, tag="sc")
                        nc.vector.tensor_add(
                            sc[:],
                            v_all[:, r - 1, :].unsqueeze(1)
                            .broadcast_to((128, K, K)),
                            c_at[:])
                        vm = fwd.tile([128, K], f32, tag="vm")
                        nc.vector.tensor_reduce(vm[:], sc[:], axis=X, op=op.max)
                        nc.vector.tensor_add(v_all[:, r, :], feats[:, r, :],
                                             vm[:])
                        if r == LF:
                            selt = fwd.tile([128, K], f32, tag="selt")
                            nc.vector.tensor_add(selt[:], feats[:, LF, :],
                                                 c_st[:])
                            selo = fwd.tile([128, K], f32, tag="selo")
                            nc.vector.tensor_sub(selo[:], selt[:],
                                                 v_all[:, LF, :])
                            nc.vector.tensor_mul(selo[:], selo[:], c_m0k[:])
                            nc.vector.tensor_add(v_all[:, LF, :],
                                                 v_all[:, LF, :], selo[:])

                # ---------- final argmax g* on (b,7) lanes ----------
                with tc.tile_pool(name="gstp", bufs=1) as gst, \
                     tc.tile_pool(name="gpsm", bufs=2, space="PSUM") as gps:
                    vfin = gst.tile([128, K], f32)
                    nc.vector.tensor_add(vfin[:], v_all[:, 143, :], c_sp[:])
                    m0 = gst.tile([128, 1], f32)
                    nc.vector.tensor_reduce(m0[:], vfin[:], axis=X, op=op.max)
                    eqg = gst.tile([128, K], f32)
                    nc.vector.tensor_tensor(eqg[:], vfin[:],
                                            m0[:].broadcast_to((128, K)),
                                            op=op.is_ge)
                    nc.vector.tensor_mul(eqg[:], eqg[:], c_c1[:, 0:K])
                    gmin = gst.tile([128, 1], f32)
                    nc.vector.tensor_reduce(gmin[:], eqg[:], axis=X, op=op.min)
                    nc.vector.tensor_scalar_add(gmin[:], gmin[:], BIG)
                    ohg = gst.tile([128, K], f32)
                    nc.vector.tensor_tensor(ohg[:], c_iot[:, 0:K],
                                            gmin[:].broadcast_to((128, K)),
                                            op=op.is_equal)
                    sptmp = gst.tile([128, K], f32)
                    nc.vector.tensor_mul(sptmp[:], ohg[:], c_sp[:])
                    spp = gst.tile([128, 1], f32)
                    nc.vector.tensor_reduce(spp[:], sptmp[:], axis=X,
                                            op=op.add)
                    nc.vector.tensor_mul(spp[:], spp[:], c_m7o[:])
                    ohgT = gps.tile([K, 128], f32)
                    nc.tensor.transpose(ohgT[:], ohg[:], c_i128[:])

                    # ---------- backtrace ----------
                    with tc.tile_pool(name="btsp", bufs=1) as bts, \
                         tc.tile_pool(name="btpp", bufs=2, space="PSUM") as btp, \
                         tc.tile_pool(name="btwp", bufs=2) as btw:
                        lhsT = []
                        for q in range(4):
                            lt = bts.tile([128, 128], f32, name=f"lhsT{q}")
                            nc.sync.dma_start(lt[:], oh0.ap())
                            lhsT.append(lt)
                        ohc = bts.tile([128, 16, NE * K], f32)
                        acol = bts.tile([128, 16, NE * K], f32)
                        for s in range(32):
                            if s == 16:
                                # overwrite tail-slot state & commit with g*
                                nc.vector.tensor_copy(
                                    lhsT[3][64:64 + K, 7:128:8],
                                    ohgT[:, 7:128:8])
                                blt = btw.tile([128, K], f32, tag="blt")
                                nc.vector.tensor_mul(
                                    blt[:], ohc[:, 15, 7 * K:NE * K],
                                    c_km7[:])
                                bl2 = btw.tile([128, K], f32, tag="bl2")
                                nc.vector.tensor_mul(
                                    bl2[:], ohg[:],
                                    c_m7o[:].broadcast_to((128, K)))
                                nc.vector.tensor_add(blt[:], blt[:], bl2[:])
                                nc.vector.tensor_copy(
                                    ohc[:, 15, 7 * K:NE * K], blt[:])
                            vrd = v_all[:, 46 - s:159 - s:16, :]  # [128,8,50]
                            ps2 = btp.tile([128, NE * K], f32, tag="ps2")
                            nc.tensor.matmul(
                                ps2.rearrange("p (e k) -> p e k", e=NE),
                                c_i128[:], vrd,
                                start=True, stop=False)
                            for q in range(4):
                                nc.tensor.matmul(
                                    ps2[:, q * 2 * K:(q + 1) * 2 * K],
                                    lhsT[q][:], c_selr[:],
                                    start=False, stop=(q == 3))
                            ps2v = ps2.rearrange("p (e k) -> p e k", e=NE)
                            vm2 = btw.tile([128, NE], f32, tag="vm2")
                            nc.vector.tensor_reduce(vm2[:], ps2v, axis=X,
                                                    op=op.max)
                            eq2 = btw.tile([128, NE, K], f32, tag="eq2")
                            nc.vector.tensor_tensor(
                                eq2[:], ps2v,
                                vm2[:].unsqueeze(2).broadcast_to((128, NE, K)),
                                op=op.is_ge)
                            nc.vector.tensor_mul(
                                eq2[:], eq2[:],
                                c_c1[:].rearrange("p (e k) -> p e k", e=NE))
                            mn2 = btw.tile([128, NE], f32, tag="mn2")
                            nc.vector.tensor_reduce(mn2[:], eq2[:], axis=X,
                                                    op=op.min)
                            nc.vector.tensor_scalar_add(mn2[:], mn2[:], BIG)
                            committed = 15 <= s < 31
                            if committed:
                                u = 30 - s
                                ohdst = ohc[:, u, :].rearrange(
                                    "p (e k) -> p e k", e=NE)
                                nc.vector.tensor_sub(
                                    acol[:, u, :].rearrange(
                                        "p (e k) -> p e k", e=NE),
                                    ps2v, vrd)
                            else:
                                ohw = btw.tile([128, NE, K], f32, tag="ohw")
                                ohdst = ohw[:]
                            nc.vector.tensor_tensor(
                                ohdst, c_iot[:].rearrange(
                                    "p (e k) -> p e k", e=NE),
                                mn2[:].unsqueeze(2).broadcast_to((128, NE, K)),
                                op=op.is_equal)
                            if s < 31:
                                ohflat = (ohc[:, 30 - s, :] if committed
                                          else ohw.rearrange(
                                              "p e k -> p (e k)"))
                                for q in range(4):
                                    for h in range(2):
                                        e_i = 2 * q + h
                                        tps = btp.tile([K, 128], f32,
                                                       tag="tps")
                                        nc.tensor.transpose(
                                            tps[:],
                                            ohflat[:, e_i * K:(e_i + 1) * K],
                                            c_i128[:])
                                        nc.vector.tensor_copy(
                                            lhsT[q][64 * h:64 * h + K, :],
                                            tps[:])

                        nc.vector.tensor_mul(acol[:, 15, 7 * K:NE * K],
                                             acol[:, 15, 7 * K:NE * K],
                                             c_km7[:])

                        # ---------- outputs ----------
                        iotv = c_iot[:].rearrange("p (e k) -> p e k", e=NE)
                        tgf = bts.tile([128, 16, NE], f32)
                        psl = bts.tile([128, 1], f32)
                        nc.vector.memset(psl[:], 0.0)
                        for u in range(16):
                            ohcu = ohc[:, u, :].rearrange(
                                "p (e k) -> p e k", e=NE)
                            w4 = btw.tile([128, NE, K], f32, tag="w4")
                            nc.vector.tensor_mul(w4[:], ohcu, iotv)
                            nc.vector.tensor_reduce(tgf[:, u, :], w4[:],
                                                    axis=X, op=op.add)
                            nc.vector.tensor_mul(
                                w4[:], ohcu,
                                feats[:, LF + u:LF + u + 113:16, :])
                            w4b = btw.tile([128, NE, K], f32, tag="w4b")
                            nc.vector.tensor_mul(
                                w4b[:], ohcu,
                                acol[:, u, :].rearrange(
                                    "p (e k) -> p e k", e=NE))
                            nc.vector.tensor_add(w4[:], w4[:], w4b[:])
                            pu = btw.tile([128, 1], f32, tag="pu")
                            nc.vector.tensor_reduce(pu[:], w4[:], axis=XY,
                                                    op=op.add)
                            nc.vector.tensor_add(psl[:], psl[:], pu[:])
                        tgi = bts.tile([128, NE, 16], i32)
                        nc.vector.tensor_copy(
                            tgi[:], tgf.rearrange("p u e -> p e u"))
                        nc.sync.dma_start(
                            tags_out.ap().rearrange(
                                "b (c e u) -> b c e u", c=C1, e=NE),
                            tgi[:])

                        stp = bts.tile([128, K], f32)
                        nc.vector.tensor_mul(stp[:], ohc[:, 0, 0:K],
                                             c_stm[:])
                        extra = bts.tile([128, 1], f32)
                        nc.vector.tensor_reduce(extra[:], stp[:], axis=X,
                                                op=op.add)
                        nc.vector.tensor_add(extra[:], extra[:], spp[:])
                        nc.vector.tensor_add(psl[:], psl[:], extra[:])
                        bst_ps = gps.tile([BC, 1], f32)
                        nc.tensor.matmul(bst_ps[:], c_grp[:], psl[:],
                                         start=True, stop=True)
                        bst = bts.tile([BC, 1], f32)
                        nc.vector.tensor_copy(bst[:], bst_ps[:])
                        nc.sync.dma_start(best_out.ap(), bst[:])

    nc.compile()
    return nc


def make_inputs(hidden, W, b, transitions, start_transitions, stop_transitions):
    hidden = np.asarray(hidden, np.float32)
    W = np.asarray(W, np.float32)
    b = np.asarray(b, np.float32)
    A = np.asarray(transitions, np.float32)
    st = np.asarray(start_transitions, np.float32)
    sp = np.asarray(stop_transitions, np.float32)

    iota = np.arange(K).astype(np.float32)
    selr = np.zeros((128, 2 * K), np.float32)
    selr[0:K, 0:K] = A.T
    selr[64:64 + K, K:2 * K] = A.T
    oh0 = np.zeros((128, 128), np.float32)
    oh0[0, :] = 1.0
    oh0[64, :] = 1.0
    grp = np.zeros((128, BC), np.float32)
    for p in range(128):
        grp[p, p // C1] = 1.0
    m0k_a = np.zeros((128, K), np.float32); m0k_a[0:121:8, :] = 1.0
    stm_a = np.zeros((128, K), np.float32); stm_a[0:121:8, :] = st
    m7o_a = np.zeros((128, 1), np.float32); m7o_a[7:128:8, :] = 1.0
    km7_a = np.ones((128, K), np.float32); km7_a[7:128:8, :] = 0.0
    common = {
        "m0k": m0k_a, "stm": stm_a, "m7o": m7o_a, "km7": km7_a,
        "wt": np.ascontiguousarray(W.T),
        "b_rep": np.broadcast_to(b, (128, K)).copy(),
        "st_rep": np.broadcast_to(st, (128, K)).copy(),
        "sp_rep": np.broadcast_to(sp, (128, K)).copy(),
        "at_rep": np.broadcast_to(A.T.reshape(1, K * K), (128, K * K)).copy(),
        "i128": np.eye(128, dtype=np.float32),
        "selr": selr,
        "c1c": np.broadcast_to(np.tile(iota, NE) - BIG, (128, NE * K)).copy(),
        "iot": np.broadcast_to(np.tile(iota, NE), (128, NE * K)).copy(),
        "oh0": oh0,
        "grp": grp,
    }
    in_maps = []
    for core in range(NCORES):
        hb = hidden[core * BC:(core + 1) * BC]
        in_maps.append(
            {"hT": np.ascontiguousarray(hb.reshape(BC * T, H).T), **common})
    return in_maps


def kernel(hidden, W, b, transitions, start_transitions, stop_transitions):
    from concourse.bass_utils import run_bass_kernel_spmd

    if "nc" not in _cache:
        _cache["nc"] = _build()
    nc = _cache["nc"]
    in_maps = make_inputs(hidden, W, b, transitions, start_transitions,
                          stop_transitions)
    res = run_bass_kernel_spmd(nc, in_maps, list(range(NCORES)))
    tags = np.concatenate([res.results[c]["tags"] for c in range(NCORES)], 0)
    best = np.concatenate(
        [res.results[c]["best"][:, 0] for c in range(NCORES)], 0)
    return best.astype(np.float32), tags.astype(np.int32)
